# revision 1
# baseline (speedup 1.0000x reference)
"""Trainium2 Bass kernel for nn_BertSVDBlock (B=8, M=1024, D=768, H=12).

Sharding: pure data-parallel over batch B — core b computes batch element b.
No collectives needed.

Device-side design (everything in transposed layout, zero on-device
transposes; host pre-transposes x and post-transposes the output):

  xT[d, m]                                  (fp32 for residual, bf16 for PE)
  tmpT = P_pack.T @ xbT                     (QKV low-rank first factors, with a
                                             "bias slot" row per 128-col group
                                             memset to 1.0 so the second
                                             factor can fold biases in)
  QT_h/KT_h[dh, m] = W2.T @ tmpT            (bq/bk folded via the ones row)
  V_h[n, dh | 1]   = tmpT_slices.T @ W2v    (natural orientation; bv folded in;
                                             an extra all-ones column makes the
                                             softmax denominator fall out of
                                             the PV matmul for free)
  scoresT_h[n, m]  = KT_h_slice.T @ QT_h    (keys on partitions)
  probsT = exp(0.125*scoresT + maskbias[n]) (single ACT pass, psum->sbuf bf16;
                                             no max-subtraction needed: scores
                                             are O(0.05) for this problem)
  A_h[dh|den, m]   = V_h.T @ probsT         (unnormalized attention + denom row)
  attn_scaled      = A_h * (1/denom)        (DVE reciprocal + DRAM-bounce
                                             partition broadcast + DVE mult)
  attn_out chain   = Vo.T @ (Uo.T @ attn_scaled),  z = attn_out + bo + xT
  LN over the partition axis via PE ones-matmul column sums;
  rsqrt as exp(-0.5*ln(var+eps)) to stay in one ACT table set.
  FFN with GELU(+b1) fused in one ACT op per tile, LN2, DMA out.
"""

import os
import sys

import numpy as np

for _p in ("/opt/trn_rl_repo", "/root/.axon_site/_ro/trn_rl_repo"):
    if os.path.isdir(_p) and _p not in sys.path:
        sys.path.append(_p)

import ml_dtypes

BF16 = ml_dtypes.bfloat16

# Problem constants (hardcoded per the harness contract).
B, M, D, H, DH = 8, 1024, 768, 12, 64
R_ATTN, R_FF, R_WO, DFF = 32, 256, 256, 3072
LN_EPS = 1e-12
N_CORES = 8
P = 128
KD = D // P           # 6 k-chunks over D
NPT = M // P          # 8 n-partition-tiles over sequence
GROUPS = 12           # 12 col-groups in P_pack (Q:0-3, K:4-7, V:8-11)
FFT = DFF // P        # 24 dff partition tiles

_prog_cache: dict = {}
last_results = None   # test.py reads exec_time_ns / profile from here


def _bcast_rows(nc, dram_pool, dst, src, nrows, ncols, tag):
    """Broadcast src [1, ncols] SBUF to dst [nrows, ncols] SBUF via a DRAM
    bounce (DRAM-source DMAs may use step-0 partition APs; SBUF ones can't).
    """
    from concourse import mybir
    dr = dram_pool.tile([1, ncols], mybir.dt.float32, tag=tag, name=tag)
    nc.sync.dma_start(out=dr, in_=src)
    nc.sync.dma_start(out=dst, in_=dr[0:1, :].to_broadcast((nrows, ncols)))


def _layernorm_T(nc, tc, mybir, zs, out_tiles, ones_col, dram_pool,
                 gain=None, bias=None):
    """LayerNorm over the partition dimension (d) of transposed tiles.

    zs: list of KD [128, M] fp32 tiles.  out_tiles: callable k -> dest AP.
    Stats via PE ones-matmul column sums; per-column a=rsqrt(var+eps) and
    c=mu*a are partition-broadcast via a DRAM bounce, applied on DVE/GPSIMD.
    rsqrt computed as exp(-0.5*ln(var+eps)) — Ln and Exp share one ACT
    table set with the softmax exp, avoiding table reloads.
    """
    from contextlib import ExitStack
    OP = mybir.AluOpType
    AF = mybir.ActivationFunctionType
    f32 = mybir.dt.float32
    bf16 = mybir.dt.bfloat16

    with ExitStack() as ctx:
        abc = ctx.enter_context(tc.tile_pool(name="ln_abc", bufs=1))
        stat = ctx.enter_context(tc.tile_pool(name="ln_stat", bufs=1))

        zb, zq = [], []
        for k in range(KD):
            zbk = abc.tile([P, M], bf16, tag=f"ln_zb{k}", name=f"zb{k}")
            zqk = abc.tile([P, M], bf16, tag=f"ln_zq{k}", name=f"zq{k}")
            nc.gpsimd.tensor_copy(out=zbk, in_=zs[k])
            if k % 2 == 0:
                nc.scalar.activation(out=zqk, in_=zs[k], func=AF.Square)
            else:
                nc.vector.tensor_tensor(out=zqk, in0=zs[k], in1=zs[k],
                                        op=OP.mult)
            zb.append(zbk)
            zq.append(zqk)

        a_sb = abc.tile([1, M], f32, tag="ln_a")
        c_sb = abc.tile([1, M], f32, tag="ln_c")
        eps_t = abc.tile([1, 1], f32, tag="ln_eps")
        nc.vector.memset(eps_t, LN_EPS)
        with tc.tile_pool(name="ps_ln", bufs=2, space="PSUM") as ps_ln:
            s1 = ps_ln.tile([1, M], f32, tag="lns")
            s2 = ps_ln.tile([1, M], f32, tag="lns")
            for mi in range(2):
                sl = slice(mi * 512, (mi + 1) * 512)
                for k in range(KD):
                    nc.tensor.matmul(s1[:, sl], ones_col, zb[k][:, sl],
                                     start=(k == 0), stop=(k == KD - 1),
                                     skip_group_check=True)
                for k in range(KD):
                    nc.tensor.matmul(s2[:, sl], ones_col, zq[k][:, sl],
                                     start=(k == 0), stop=(k == KD - 1),
                                     skip_group_check=True)
            mu = stat.tile([1, M], f32, tag="ln_mu")
            var = stat.tile([1, M], f32, tag="ln_var")
            musq = stat.tile([1, M], f32, tag="ln_musq")
            nc.vector.tensor_scalar_mul(mu, s1, 1.0 / D)
            nc.vector.tensor_tensor(out=musq, in0=mu, in1=mu, op=OP.mult)
            nc.vector.scalar_tensor_tensor(
                out=var, in0=s2, scalar=1.0 / D, in1=musq,
                op0=OP.mult, op1=OP.subtract)
            # a = 1/sqrt(var+eps) = exp(-0.5*ln(var+eps));
            # Ln/Exp share the softmax-exp ACT table set (no reload)
            lnv = stat.tile([1, M], f32, tag="ln_lnv")
            nc.scalar.activation(out=lnv, in_=var, func=AF.Ln,
                                 bias=eps_t, scale=1.0)
            nc.scalar.activation(out=a_sb, in_=lnv, func=AF.Exp,
                                 scale=-0.5)
            nc.vector.tensor_tensor(out=c_sb, in0=mu, in1=a_sb, op=OP.mult)

        a_b = abc.tile([P, M], f32, tag="ln_ab")
        c_b = abc.tile([P, M], f32, tag="ln_cb")
        _bcast_rows(nc, dram_pool, a_b, a_sb, P, M, "ln_ab_dr")
        _bcast_rows(nc, dram_pool, c_b, c_sb, P, M, "ln_cb_dr")

        for k in range(KD):
            t1 = abc.tile([P, M], f32, tag="ln_t1", bufs=4)
            dst = out_tiles(k)
            nc.gpsimd.tensor_tensor(out=t1, in0=zs[k], in1=a_b, op=OP.mult)
            if gain is None and bias is None:
                nc.vector.tensor_tensor(out=dst, in0=t1, in1=c_b,
                                        op=OP.subtract)
            else:
                nc.vector.tensor_tensor(out=t1, in0=t1, in1=c_b,
                                        op=OP.subtract)
                gk = gain[:, k:k + 1] if gain is not None else 1.0
                if bias is not None:
                    bb = bias[:, k:k + 1].to_broadcast((P, M))
                    nc.vector.scalar_tensor_tensor(
                        out=dst, in0=t1, scalar=gk, in1=bb,
                        op0=OP.mult, op1=OP.add)
                else:
                    nc.vector.tensor_scalar_mul(dst, t1, gk)


def _build_program(has_aff1: bool, has_aff2: bool):
    """Build the SPMD Bass program (same program runs on all 8 cores)."""
    from contextlib import ExitStack

    import concourse.bass as bass
    import concourse.tile as tile
    from concourse import bacc
    from concourse import mybir

    f32 = mybir.dt.float32
    bf16 = mybir.dt.bfloat16
    AF = mybir.ActivationFunctionType
    OP = mybir.AluOpType

    nc = bacc.Bacc("TRN2", target_bir_lowering=False)

    # ---- I/O declarations (names are the in_map keys) ----
    xT_d = nc.dram_tensor("xT", [D, M], f32, kind="ExternalInput")
    xb_d = nc.dram_tensor("xb", [D, M], bf16, kind="ExternalInput")
    pp_d = nc.dram_tensor("p_pack", [D, GROUPS * P], bf16, kind="ExternalInput")
    w2q_d = nc.dram_tensor("w2q", [P, H, DH], bf16, kind="ExternalInput")
    w2k_d = nc.dram_tensor("w2k", [P, H, DH], bf16, kind="ExternalInput")
    w2v_d = nc.dram_tensor("w2v", [P, H, DH + 1], bf16, kind="ExternalInput")
    uo_d = nc.dram_tensor("uo", [D, R_WO], bf16, kind="ExternalInput")
    vo_d = nc.dram_tensor("vo", [R_WO, D], bf16, kind="ExternalInput")
    u1_d = nc.dram_tensor("u1", [D, R_FF], bf16, kind="ExternalInput")
    v1_d = nc.dram_tensor("v1", [R_FF, DFF], bf16, kind="ExternalInput")
    u2_d = nc.dram_tensor("u2", [DFF, R_FF], bf16, kind="ExternalInput")
    v2_d = nc.dram_tensor("v2", [R_FF, D], bf16, kind="ExternalInput")
    b1_d = nc.dram_tensor("b1c", [DFF], f32, kind="ExternalInput")
    bo_d = nc.dram_tensor("boc", [D], f32, kind="ExternalInput")
    b2_d = nc.dram_tensor("b2c", [D], f32, kind="ExternalInput")
    mb_d = nc.dram_tensor("maskb", [M], f32, kind="ExternalInput")
    ln_d = {}
    if has_aff1:
        ln_d["g1"] = nc.dram_tensor("lng1", [D], f32, kind="ExternalInput")
        ln_d["b1"] = nc.dram_tensor("lnb1", [D], f32, kind="ExternalInput")
    if has_aff2:
        ln_d["g2"] = nc.dram_tensor("lng2", [D], f32, kind="ExternalInput")
        ln_d["b2"] = nc.dram_tensor("lnb2", [D], f32, kind="ExternalInput")
    out_d = nc.dram_tensor("outT", [D, M], f32, kind="ExternalOutput")

    with ExitStack() as top:
        tc = top.enter_context(tile.TileContext(nc))
        dma = nc.sync.dma_start

        consts = top.enter_context(tc.tile_pool(name="consts", bufs=1))
        dram_pool = top.enter_context(
            tc.tile_pool(name="drb", bufs=6, space="DRAM"))
        z1p = top.enter_context(tc.tile_pool(name="z1p", bufs=1))

        ones_col = consts.tile([P, 1], bf16, name="ones_col")
        nc.vector.memset(ones_col, 1.0)
        b1c = consts.tile([P, FFT], f32, name="b1c")
        boc = consts.tile([P, KD], f32, name="boc")
        b2c = consts.tile([P, KD], f32, name="b2c")
        maskb = consts.tile([P, NPT], f32, name="maskb")
        aff = {}
        for key, dd in ln_d.items():
            aff[key] = consts.tile([P, KD], f32, name="aff_" + key)

        def _dma_consts():
            dma(maskb, mb_d.rearrange("(j p) -> p j", p=P))
            dma(b1c, b1_d.rearrange("(k p) -> p k", p=P))
            dma(boc, bo_d.rearrange("(k p) -> p k", p=P))
            dma(b2c, b2_d.rearrange("(k p) -> p k", p=P))
            for key, dd in ln_d.items():
                dma(aff[key], dd.rearrange("(k p) -> p k", p=P))

        # ======== big1 scope: QKV + attention + out-proj ========
        with ExitStack() as big1:
            bigp = big1.enter_context(tc.tile_pool(name="big1", bufs=1))
            # per-k attention output (heads 2k, 2k+1 -> partition halves)
            attn_sc = [bigp.tile([P, M], bf16, name=f"attn_sc{k}")
                       for k in range(KD)]

            with ExitStack() as ph12:
                pA = ph12.enter_context(tc.tile_pool(name="pA", bufs=1))
                probs_pool = ph12.enter_context(
                    tc.tile_pool(name="probs", bufs=8))
                small_pool = ph12.enter_context(
                    tc.tile_pool(name="small", bufs=4))

                w2q = pA.tile([P, H, DH], bf16, name="w2q")
                w2k = pA.tile([P, H, DH], bf16, name="w2k")
                w2v = pA.tile([P, H, DH + 1], bf16, name="w2v")
                tmp = pA.tile([P, GROUPS, M], bf16, name="tmp")
                qb = pA.tile([P, H // 2, M], bf16, name="qb")
                kb = pA.tile([P, H // 2, M], bf16, name="kb")
                vb = pA.tile([P, H, NPT * (DH + 1)], bf16, name="vb")

                # ---- Phase 1a: QKV first factor ----
                with ExitStack() as ph1:
                    pAA = ph1.enter_context(tc.tile_pool(name="pAA", bufs=1))
                    xb = pAA.tile([P, KD, M], bf16, name="xbt")
                    xb_r = xb_d.rearrange("(k p) m -> p k m", p=P)
                    p_pack = pAA.tile([P, KD, GROUPS * P], bf16, name="p_pack")
                    pp_r = pp_d.rearrange("(k p) c -> p k c", p=P)
                    # critical-path tensors stream first, k-interleaved so
                    # the k=0 matmuls can start asap; weights/consts follow
                    for k in range(KD):
                        dma(xb[:, k, :], xb_r[:, k, :])
                        dma(p_pack[:, k, :], pp_r[:, k, :])
                    dma(w2q, w2q_d[:])
                    dma(w2k, w2k_d[:])
                    dma(w2v, w2v_d[:])
                    _dma_consts()

                    with tc.tile_pool(name="ps1", bufs=4,
                                      space="PSUM") as ps_ff:
                        for g in range(GROUPS):
                            ps = ps_ff.tile([P, M], f32, tag="ff")
                            for k in range(KD):
                                for mi in range(2):
                                    nc.tensor.matmul(
                                        ps[:, mi * 512:(mi + 1) * 512],
                                        p_pack[:, k, g * P:(g + 1) * P],
                                        xb[:, k, mi * 512:(mi + 1) * 512],
                                        start=(k == 0), stop=(k == KD - 1),
                                        skip_group_check=True,
                                    )
                            if g % 2 == 0:
                                nc.vector.tensor_copy(out=tmp[:, g, :],
                                                      in_=ps)
                            else:
                                nc.scalar.copy(out=tmp[:, g, :], in_=ps)
                            # bias-slot row -> 1.0 (folds biases into the
                            # second-factor matmuls)
                            nc.vector.memset(tmp[96:97, g, :], 1.0)

                # ---- Phase 1b: QKV second factors ----
                # QK evacuations ride the otherwise-idle ACT engine here.
                with tc.tile_pool(name="ps1qk", bufs=3, space="PSUM") as ps_qk:
                    for h in range(H):
                        po = 64 * (h % 2)
                        for (w2, dst, goff) in ((w2q, qb, 0), (w2k, kb, 4)):
                            ps = ps_qk.tile([DH, M], f32, tag="qk")
                            for mi in range(2):
                                nc.tensor.matmul(
                                    ps[:, mi * 512:(mi + 1) * 512],
                                    w2[:, h, :],
                                    tmp[:, goff + h // 3,
                                        mi * 512:(mi + 1) * 512],
                                    start=True, stop=True,
                                    skip_group_check=True,
                                )
                            if h % 2 == 0:
                                nc.vector.tensor_copy(
                                    out=dst[po:po + DH, h // 2, :], in_=ps)
                            else:
                                nc.scalar.copy(
                                    out=dst[po:po + DH, h // 2, :], in_=ps)

                with tc.tile_pool(name="ps1v", bufs=6, space="PSUM") as ps_v:
                    for g in range(4):
                        pss = [ps_v.tile([P, 4 * (DH + 1)], f32, tag="v",
                                         name=f"psv_{g}_{i}")
                               for i in range(6)]
                        for j in range(NPT):
                            lhsT = tmp[:, 8 + g, j * P:(j + 1) * P]
                            for hh in range(3):
                                ps = pss[hh * 2 + j // 4]
                                nc.tensor.matmul(
                                    ps[:, (j % 4) * (DH + 1):
                                       (j % 4 + 1) * (DH + 1)],
                                    lhsT, w2v[:, 3 * g + hh, :],
                                    start=True, stop=True,
                                    skip_group_check=True,
                                )
                        for hh in range(3):
                            h = 3 * g + hh
                            for half in range(2):
                                eng = nc.vector.tensor_copy if half == 0 \
                                    else nc.scalar.copy
                                if half == 0:
                                    nc.vector.tensor_copy(
                                        out=vb[:, h, half * 4 * (DH + 1):
                                               (half + 1) * 4 * (DH + 1)],
                                        in_=pss[hh * 2 + half])
                                else:
                                    nc.scalar.copy(
                                        out=vb[:, h, half * 4 * (DH + 1):
                                               (half + 1) * 4 * (DH + 1)],
                                        in_=pss[hh * 2 + half])

                # ---- Phase 2: attention ----
                with tc.tile_pool(name="ps2sc", bufs=2, space="PSUM") as ps_sc, \
                     tc.tile_pool(name="ps2at", bufs=2, space="PSUM") as ps_at:
                    for h in range(H):
                        po = 64 * (h % 2)
                        slq = h // 2
                        at = ps_at.tile([DH + 1, M], f32, tag="at")
                        for j in range(NPT):
                            sc = ps_sc.tile([P, M], f32, tag="sc")
                            for mi in range(2):
                                nc.tensor.matmul(
                                    sc[:, mi * 512:(mi + 1) * 512],
                                    kb[po:po + DH, slq, j * P:(j + 1) * P],
                                    qb[po:po + DH, slq,
                                       mi * 512:(mi + 1) * 512],
                                    start=True, stop=True,
                                    skip_group_check=True,
                                )
                            pr = probs_pool.tile([P, M], bf16, tag="probs")
                            nc.scalar.activation(
                                out=pr, in_=sc, func=AF.Exp,
                                bias=maskb[:, j:j + 1], scale=0.125)
                            for mi in range(2):
                                nc.tensor.matmul(
                                    at[:, mi * 512:(mi + 1) * 512],
                                    vb[:, h, j * (DH + 1):(j + 1) * (DH + 1)],
                                    pr[:, mi * 512:(mi + 1) * 512],
                                    start=(j == 0), stop=(j == NPT - 1),
                                    skip_group_check=True,
                                )
                        # normalize: attn = A / denom
                        rec = small_pool.tile([1, M], f32, tag="rec")
                        rb = small_pool.tile([DH, M], f32, tag="rb")
                        nc.vector.reciprocal(out=rec, in_=at[DH:DH + 1, :])
                        _bcast_rows(nc, dram_pool, rb, rec, DH, M, "rec_dr")
                        nc.vector.tensor_tensor(
                            out=attn_sc[slq][po:po + DH, :],
                            in0=at[0:DH, :], in1=rb, op=OP.mult)

            # ---- Phase 3: output projection (+ late fp32 x DMA) ----
            xT = [bigp.tile([P, M], f32, name=f"xT{k}") for k in range(KD)]
            z1 = [z1p.tile([P, M], f32, name=f"z1_{k}") for k in range(KD)]
            with ExitStack() as ph3:
                pB = ph3.enter_context(tc.tile_pool(name="pB", bufs=1))
                uo = pB.tile([P, KD, R_WO], bf16, name="uo")
                dma(uo, uo_d.rearrange("(k p) c -> p k c", p=P))
                vo = pB.tile([P, 2, D], bf16, name="vo")
                dma(vo, vo_d.rearrange("(k p) c -> p k c", p=P))
                for k in range(KD):
                    dma(xT[k], xT_d[k * P:(k + 1) * P, :])
                h1b = pB.tile([P, 2, M], bf16, name="h1b")
                with tc.tile_pool(name="ps3h", bufs=2, space="PSUM") as ps_h1, \
                     tc.tile_pool(name="ps3v", bufs=2, space="PSUM") as ps_vo:
                    for pt in range(2):
                        for mi in range(2):
                            ps = ps_h1.tile([P, 512], f32, tag="h1")
                            for k in range(KD):
                                nc.tensor.matmul(
                                    ps,
                                    uo[:, k, pt * P:(pt + 1) * P],
                                    attn_sc[k][:, mi * 512:(mi + 1) * 512],
                                    start=(k == 0), stop=(k == KD - 1),
                                )
                            if mi == 0:
                                nc.vector.tensor_copy(
                                    out=h1b[:, pt, mi * 512:(mi + 1) * 512],
                                    in_=ps)
                            else:
                                nc.scalar.copy(
                                    out=h1b[:, pt, mi * 512:(mi + 1) * 512],
                                    in_=ps)
                    for k in range(KD):
                        ps = ps_vo.tile([P, M], f32, tag="voo")
                        for r in range(2):
                            for mi in range(2):
                                nc.tensor.matmul(
                                    ps[:, mi * 512:(mi + 1) * 512],
                                    vo[:, r, k * P:(k + 1) * P],
                                    h1b[:, r, mi * 512:(mi + 1) * 512],
                                    start=(r == 0), stop=(r == 1),
                                    skip_group_check=True,
                                )
                        # z = attn_out + bo + x
                        nc.vector.scalar_tensor_tensor(
                            out=z1[k], in0=ps, scalar=boc[:, k:k + 1],
                            in1=xT[k], op0=OP.add, op1=OP.add)

        # ---- FFN weight prefetch (overlaps LN1) ----
        ffw = top.enter_context(tc.tile_pool(name="ffw", bufs=1))
        u1 = ffw.tile([P, KD, R_FF], bf16, name="u1")
        dma(u1, u1_d.rearrange("(k p) c -> p k c", p=P))
        v1 = ffw.tile([P, 2, DFF], bf16, name="v1")
        dma(v1, v1_d.rearrange("(k p) c -> p k c", p=P))
        u2 = ffw.tile([P, FFT, R_FF], bf16, name="u2")
        dma(u2, u2_d.rearrange("(k p) c -> p k c", p=P))
        v2 = ffw.tile([P, 2, D], bf16, name="v2")
        dma(v2, v2_d.rearrange("(k p) c -> p k c", p=P))

        # ---- LN1 (consumes z1, writes x1 fp32 + x1b bf16) ----
        x1_pool = top.enter_context(tc.tile_pool(name="x1p", bufs=1))
        x1 = [x1_pool.tile([P, M], f32, name=f"x1_{k}") for k in range(KD)]
        x1b = [x1_pool.tile([P, M], bf16, name=f"x1b_{k}") for k in range(KD)]
        _layernorm_T(nc, tc, mybir, z1, lambda k: x1[k],
                     ones_col, dram_pool,
                     gain=aff.get("g1"), bias=aff.get("b1"))
        for k in range(KD):
            nc.gpsimd.tensor_copy(out=x1b[k], in_=x1[k])

        # ======== big2 scope: FFN + LN2 ========
        with ExitStack() as big2:
            big2p = big2.enter_context(tc.tile_pool(name="big2", bufs=1))
            z2 = [big2p.tile([P, M], f32, name=f"z2_{k}") for k in range(KD)]

            with ExitStack() as ph4w:
                pCw = ph4w.enter_context(tc.tile_pool(name="pCw", bufs=1))
                g2b = pCw.tile([P, 2, M], bf16, name="g2b")

                with ExitStack() as phff:
                    pC1 = phff.enter_context(tc.tile_pool(name="pC1", bufs=1))
                    midb = pC1.tile([P, 2, M], bf16, name="midb")
                    dffb = pC1.tile([P, FFT, M], bf16, name="dffb")
                    with tc.tile_pool(name="ps4m", bufs=2,
                                      space="PSUM") as ps_mid:
                        for pt in range(2):
                            for mi in range(2):
                                ps = ps_mid.tile([P, 512], f32, tag="mid")
                                for k in range(KD):
                                    nc.tensor.matmul(
                                        ps,
                                        u1[:, k, pt * P:(pt + 1) * P],
                                        x1b[k][:, mi * 512:(mi + 1) * 512],
                                        start=(k == 0), stop=(k == KD - 1),
                                    )
                                if mi == 0:
                                    nc.vector.tensor_copy(
                                        out=midb[:, pt,
                                                 mi * 512:(mi + 1) * 512],
                                        in_=ps)
                                else:
                                    nc.scalar.copy(
                                        out=midb[:, pt,
                                                 mi * 512:(mi + 1) * 512],
                                        in_=ps)

                    with tc.tile_pool(name="ps4d", bufs=2,
                                      space="PSUM") as ps_dff, \
                         tc.tile_pool(name="ps4g", bufs=4,
                                      space="PSUM") as ps_g2:
                        for ft in range(FFT):
                            ps = ps_dff.tile([P, M], f32, tag="dff")
                            for r in range(2):
                                for mi in range(2):
                                    nc.tensor.matmul(
                                        ps[:, mi * 512:(mi + 1) * 512],
                                        v1[:, r, ft * P:(ft + 1) * P],
                                        midb[:, r, mi * 512:(mi + 1) * 512],
                                        start=(r == 0), stop=(r == 1),
                                        skip_group_check=True,
                                    )
                            # GELU(dff + b1) in one ACT pass, psum -> bf16
                            nc.scalar.activation(
                                out=dffb[:, ft, :], in_=ps, func=AF.Gelu,
                                bias=b1c[:, ft:ft + 1], scale=1.0)

                        for pt in range(2):
                            pss = [ps_g2.tile([P, 512], f32, tag="g2",
                                              name=f"g2_{pt}_{i}")
                                   for i in range(2)]
                            for ft in range(FFT):
                                for mi in range(2):
                                    nc.tensor.matmul(
                                        pss[mi],
                                        u2[:, ft, pt * P:(pt + 1) * P],
                                        dffb[:, ft, mi * 512:(mi + 1) * 512],
                                        start=(ft == 0), stop=(ft == FFT - 1),
                                    )
                            for mi in range(2):
                                nc.vector.tensor_copy(
                                    out=g2b[:, pt, mi * 512:(mi + 1) * 512],
                                    in_=pss[mi])

                with tc.tile_pool(name="ps4y", bufs=2, space="PSUM") as ps_y:
                    for k in range(KD):
                        ps = ps_y.tile([P, M], f32, tag="y")
                        for r in range(2):
                            for mi in range(2):
                                nc.tensor.matmul(
                                    ps[:, mi * 512:(mi + 1) * 512],
                                    v2[:, r, k * P:(k + 1) * P],
                                    g2b[:, r, mi * 512:(mi + 1) * 512],
                                    start=(r == 0), stop=(r == 1),
                                    skip_group_check=True,
                                )
                        nc.vector.scalar_tensor_tensor(
                            out=z2[k], in0=ps, scalar=b2c[:, k:k + 1],
                            in1=x1[k], op0=OP.add, op1=OP.add)

            # ---- LN2 + store ----
            with tc.tile_pool(name="outp", bufs=3) as out_pool:
                out_tiles = {}

                def ln2_out(k):
                    t = out_pool.tile([P, M], f32, tag="out",
                                      name=f"out_{k}")
                    out_tiles[k] = t
                    return t

                _layernorm_T(nc, tc, mybir, z2, ln2_out, ones_col,
                             dram_pool,
                             gain=aff.get("g2"), bias=aff.get("b2"))
                for k in range(KD):
                    dma(out_d[k * P:(k + 1) * P, :], out_tiles[k])

    nc.compile()
    return nc


def _prep_inputs(x, mask, Pq, Vq, bq, Pk, Vk, bk, Pv, Vv, bv,
                 Uo, Vo, bo_attn, U1, V1, b1, U2, V2, b2,
                 ln1_g, ln1_b, ln2_g, ln2_b):
    """Host-side packing: per-core in_maps for the SPMD kernel."""
    # P_pack [768, 1536]: 12 col groups of 128 (Q:0-3, K:4-7, V:8-11), each
    # [3 heads x 32 | bias-slot col 96 (zero; memset to 1 on device) | pad]
    p_pack = np.zeros((D, GROUPS * P), np.float32)
    for t, Pw in enumerate((Pq, Pk, Pv)):
        for h in range(H):
            g = t * 4 + h // 3
            c0 = g * P + 32 * (h % 3)
            p_pack[:, c0:c0 + 32] = Pw[h]
    p_pack = p_pack.astype(BF16)

    def second_factor(Vw, bw, aug):
        w = np.zeros((P, H, DH + (1 if aug else 0)), np.float32)
        for h in range(H):
            r0 = 32 * (h % 3)
            w[r0:r0 + 32, h, :DH] = Vw[h]
            w[96, h, :DH] = bw[0, h, 0, :]
            if aug:
                w[96, h, DH] = 1.0
        return w.astype(BF16)

    w2q = second_factor(Vq, bq, False)
    w2k = second_factor(Vk, bk, False)
    w2v = second_factor(Vv, bv, True)

    common = {
        "p_pack": p_pack, "w2q": w2q, "w2k": w2k, "w2v": w2v,
        "uo": Uo.astype(BF16), "vo": Vo.astype(BF16),
        "u1": U1.astype(BF16), "v1": V1.astype(BF16),
        "u2": U2.astype(BF16), "v2": V2.astype(BF16),
        "b1c": np.ascontiguousarray(b1, np.float32),
        "boc": np.ascontiguousarray(bo_attn, np.float32),
        "b2c": np.ascontiguousarray(b2, np.float32),
    }
    has_aff1 = not (np.all(ln1_g == 1.0) and np.all(ln1_b == 0.0))
    has_aff2 = not (np.all(ln2_g == 1.0) and np.all(ln2_b == 0.0))
    if has_aff1:
        common["lng1"] = np.ascontiguousarray(ln1_g, np.float32)
        common["lnb1"] = np.ascontiguousarray(ln1_b, np.float32)
    if has_aff2:
        common["lng2"] = np.ascontiguousarray(ln2_g, np.float32)
        common["lnb2"] = np.ascontiguousarray(ln2_b, np.float32)

    in_maps = []
    for b in range(B):
        m = dict(common)
        xt = np.ascontiguousarray(x[b].T, np.float32)
        m["xT"] = xt
        m["xb"] = xt.astype(BF16)
        m["maskb"] = np.where(mask[b] > 0, 0.0, -1e9).astype(np.float32)
        in_maps.append(m)
    return in_maps, has_aff1, has_aff2


def build_program_for_inputs(**inputs):
    """Build (or fetch cached) program + per-core in_maps, without running."""
    inputs = {k: np.asarray(v) for k, v in inputs.items()}
    in_maps, has_aff1, has_aff2 = _prep_inputs(**inputs)
    key = (has_aff1, has_aff2)
    if key not in _prog_cache:
        _prog_cache[key] = _build_program(has_aff1, has_aff2)
    return _prog_cache[key], in_maps


def kernel(**inputs):
    global last_results
    nc, in_maps = build_program_for_inputs(**inputs)
    from concourse.bass_utils import run_bass_kernel_spmd
    res = run_bass_kernel_spmd(nc, in_maps, list(range(N_CORES)))
    last_results = res
    out = np.stack([res.results[b]["outT"].T for b in range(B)])
    return np.ascontiguousarray(out, np.float32)



# revision 31
# speedup vs baseline: 266.9943x; 266.9943x over previous
"""Trainium2 Bass kernel for nn_BertSVDBlock (B=8, M=1024, D=768, H=12).

Sharding: pure data-parallel over batch B - core b computes batch element b.
No collectives needed.

v2 redesign (vs the v1 transposed-everything kernel):
  - Tight rank packing: 9 first-factor groups of exactly 128 (4 heads x
    rank-32 each; order K|Q|V), no ones/bias rows -> 55k PE col-cycles
    instead of 74k.
  - G-form attention scores: scoresT = ak^T (G aq + Wk bq) with
    G = Wk Wq^T (= Vk Vq^T per head) precomputed on host, so K/Q second
    factors are never materialized.  bk and all per-query-constant terms
    are dropped (softmax-invariant); bq enters via a per-partition ACT
    bias at the qg PSUM evacuation; bv is folded into bo on host
    (bo_eff = bo + concat_h(bv_h) @ Uo @ Vo, exact).
  - PV in natural orientation: out[m, dh+1] per (head, m-block) with
    probsT chunks as lhsT - half the PE column-cycles of the transposed
    form.  The extra denominator column comes from a preset ones column
    in vb.  Normalization is a per-partition reciprocal scalar multiply
    (no partition broadcasts), and the [m,d] -> [d,m] transpose back is
    done by the DMA XBAR (dma_start_transpose), costing no engine time.
  - LayerNorm stat broadcast via a PE ones outer-product into PSUM
    (no DRAM-bounce broadcasts anywhere).
  - The ACT engine is the attention bottleneck (96 exp tiles); the head
    loop is software-pipelined with all remaining first/second-factor
    work interleaved as PE filler so the PE stays busy and ramped while
    ACT grinds through the exps.
"""

import os
import sys

from collections import deque

import numpy as np

for _p in ("/opt/trn_rl_repo", "/root/.axon_site/_ro/trn_rl_repo"):
    if os.path.isdir(_p) and _p not in sys.path:
        sys.path.append(_p)

import ml_dtypes

BF16 = ml_dtypes.bfloat16
FP8 = ml_dtypes.float8_e4m3

# Problem constants (hardcoded per the harness contract).
B, M, D, H, DH = 8, 1024, 768, 12, 64
R_ATTN, R_FF, R_WO, DFF = 32, 256, 256, 3072
LN_EPS = 1e-12
N_CORES = 8
P = 128
KD = D // P           # 6 k-chunks over D
NPT = M // P          # 8 n/m-partition-blocks over sequence
NG = 3                # groups per QKV kind (4 heads x rank-32 = 128 each)
FFT = DFF // P        # 24 dff partition tiles

_prog_cache: dict = {}
last_results = None   # test.py reads exec_time_ns / profile from here


def _build_program(has_aff1: bool, has_aff2: bool,
                   b1_zero: bool = True, n_iters: int = 1):
    """Build the SPMD Bass program (same program runs on all 8 cores)."""
    from contextlib import ExitStack

    import concourse.bass as bass
    import concourse.tile as tile
    from concourse import bacc
    from concourse import mybir

    f32 = mybir.dt.float32
    bf16 = mybir.dt.bfloat16
    fp8 = mybir.dt.float8e4
    AF = mybir.ActivationFunctionType
    OP = mybir.AluOpType

    nc = bacc.Bacc("TRN2", target_bir_lowering=False)

    # ---- I/O declarations (names are the in_map keys) ----
    xb_d = nc.dram_tensor("xb", [D, M], bf16, kind="ExternalInput")
    xf_d = nc.dram_tensor("xf8", [D, M], fp8, kind="ExternalInput")
    pp_d = nc.dram_tensor("p_pack", [D, 3 * NG * P], fp8,
                          kind="ExternalInput")
    aw_d = nc.dram_tensor("awpack", [P, NG * P + H * DH], bf16,
                          kind="ExternalInput")
    uo_d = nc.dram_tensor("uo", [D, R_WO], bf16, kind="ExternalInput")
    vo_d = nc.dram_tensor("vo", [R_WO, D], bf16, kind="ExternalInput")
    u1_d = nc.dram_tensor("u1", [D, R_FF], bf16, kind="ExternalInput")
    v1_d = nc.dram_tensor("v1", [R_FF, DFF], fp8, kind="ExternalInput")
    u2_d = nc.dram_tensor("u2", [DFF, R_FF], fp8, kind="ExternalInput")
    v2_d = nc.dram_tensor("v2", [R_FF, D], bf16, kind="ExternalInput")
    cp_d = nc.dram_tensor("cpack", [P, FFT + 2 * KD + NG + 2], f32,
                          kind="ExternalInput")
    mb_d = nc.dram_tensor("maskb", [M], f32, kind="ExternalInput")
    ln_d = {}
    if has_aff1:
        ln_d["g1"] = nc.dram_tensor("lng1", [D], f32, kind="ExternalInput")
        ln_d["b1"] = nc.dram_tensor("lnb1", [D], f32, kind="ExternalInput")
    if has_aff2:
        ln_d["g2"] = nc.dram_tensor("lng2", [D], f32, kind="ExternalInput")
        ln_d["b2"] = nc.dram_tensor("lnb2", [D], f32, kind="ExternalInput")
    out_d = nc.dram_tensor("outT", [D, M], bf16, kind="ExternalOutput")

    with ExitStack() as top:
        tc = top.enter_context(tile.TileContext(nc))
        for _iter in range(n_iters):
            _emit_iteration(nc, tc, mybir, has_aff1, has_aff2, b1_zero,
                            xb_d, xf_d, pp_d, aw_d, uo_d,
                            vo_d, u1_d, v1_d, u2_d, v2_d, cp_d,
                            mb_d, ln_d, out_d)

    nc.compile()
    return nc


def _emit_iteration(nc, tc, mybir, has_aff1, has_aff2, b1_zero,
                    xb_d, xf_d, pp_d, aw_d, uo_d,
                    vo_d, u1_d, v1_d, u2_d, v2_d, cp_d,
                    mb_d, ln_d, out_d):
    from contextlib import ExitStack

    f32 = mybir.dt.float32
    bf16 = mybir.dt.bfloat16
    fp8 = mybir.dt.float8e4
    DR = mybir.MatmulPerfMode.DoubleRow
    AF = mybir.ActivationFunctionType
    OP = mybir.AluOpType
    dma = nc.sync.dma_start

    with ExitStack() as it:
        consts = it.enter_context(tc.tile_pool(name="consts", bufs=1))
        wpool = it.enter_context(tc.tile_pool(name="wpool", bufs=1))
        xpool = it.enter_context(tc.tile_pool(name="xpool", bufs=1))
        anp = it.enter_context(tc.tile_pool(name="anp", bufs=1))

        ones_col = consts.tile([P, 1], bf16, name="ones_col")
        nc.vector.memset(ones_col, 1.0)
        ones_row = consts.tile([1, P], bf16, name="ones_row")
        nc.vector.memset(ones_row, 1.0)
        eps_t = consts.tile([1, 1], f32, name="eps_t")
        nc.vector.memset(eps_t, LN_EPS)
        cpk = consts.tile([P, FFT + 2 * KD + NG + 2], f32, name="cpack")
        b1c = cpk[:, 0:FFT]
        boc = cpk[:, FFT:FFT + KD]
        b2c = cpk[:, FFT + KD:FFT + 2 * KD]
        qbias = cpk[:, FFT + 2 * KD:FFT + 2 * KD + NG]
        u1sneg = cpk[:, FFT + 2 * KD + NG:FFT + 2 * KD + NG + 2]
        maskb = consts.tile([P, NPT], f32, name="maskb")
        aff = {}
        for key, dd in ln_d.items():
            aff[key] = consts.tile([P, KD], f32, name="aff_" + key)
        dma(cpk, cp_d[:])
        dma(maskb, mb_d.rearrange("(j p) -> p j", p=P))

        # normalized attention output, transposed layout [d, m]; heads
        # 2k, 2k+1 stacked in partition halves of tile k (consumed by h1)
        attn_sc = [anp.tile([P, M], bf16, name=f"attn_sc{k}")
                   for k in range(KD)]
        # x1b (bf16) lives from LN1 through LN2 / FFN; it is both the
        # FFN input and the second residual (bf16 rounding of the LN output
        # costs ~0.4% which is well inside the 2e-2 gate)
        x1b = [xpool.tile([P, M], bf16, name=f"x1b_{k}") for k in range(KD)]
        midb = xpool.tile([P, 2, M], fp8, name="midb")
        # xb doubles as the bf16 residual input (saves the f32 x load)
        xb = xpool.tile([P, KD, M], bf16, name="xb")

        # ================= attention superscope =================
        with ExitStack() as attn:
            ap = attn.enter_context(tc.tile_pool(name="ap", bufs=1))
            prp = attn.enter_context(tc.tile_pool(name="prp", bufs=8))
            recp = attn.enter_context(tc.tile_pool(name="recp", bufs=2))
            ffps = attn.enter_context(
                tc.tile_pool(name="ffps", bufs=1, space="PSUM"))
            scps = attn.enter_context(
                tc.tile_pool(name="scps", bufs=2, space="PSUM"))
            atps = attn.enter_context(
                tc.tile_pool(name="atps", bufs=1, space="PSUM"))
            bcps = attn.enter_context(
                tc.tile_pool(name="bcps", bufs=1, space="PSUM"))

            p_pack = ap.tile([P, KD, 3 * NG * P], fp8, name="p_pack")
            xf8 = ap.tile([P, KD, M], fp8, name="xf8")
            awp = ap.tile([P, NG * P + H * DH], bf16, name="awpack")
            gpack = awp[:, 0:NG * P].rearrange("p (g c) -> p g c", c=P)
            w2v = awp[:, NG * P:].rearrange("p (h c) -> p h c", c=DH)
            tmpK = ap.tile([P, NG, M], bf16, name="tmpK")
            tmpQ = ap.tile([P, NG, M], bf16, name="tmpQ")
            tmpV = ap.tile([P, NG, M], bf16, name="tmpV")
            qgb = ap.tile([P, NG, M], bf16, name="qgb")
            tmpK3 = ap.tile([R_ATTN, NG, M], bf16, name="tmpK3")
            tmpV3 = ap.tile([R_ATTN, NG, M], bf16, name="tmpV3")
            qgb3 = ap.tile([R_ATTN, NG, M], bf16, name="qgb3")
            vb = ap.tile([P, H, NPT, DH + 1], bf16, name="vb")

            xb_r = xb_d.rearrange("(k p) m -> p k m", p=P)
            xf_r = xf_d.rearrange("(k p) m -> p k m", p=P)
            pp_r = pp_d.rearrange("(k p) c -> p k c", p=P)
            dma(xf8, xf_r)
            dma(p_pack, pp_r)
            dma(awp, aw_d[:])
            for key, dd in ln_d.items():
                dma(aff[key], dd.rearrange("(k p) -> p k", p=P))
            # preset the denominator ones-column of vb
            nc.vector.memset(vb[:, :, :, DH:DH + 1], 1.0)

            KCOL, QCOL, VCOL = 0, NG * P, 2 * NG * P

            def gen_ff(dst, col0, evac_eng, spill=None):
                """First-factor group: tmp[:, g] = p_pack_cols^T @ xb.

                Emitted as an atomic unit (psum alloc ... evac contiguous in
                program order) so its evac never waits on later PE work.
                spill: partition-0 copy of rows 96:128 (head a=3) since the
                PE cannot address base partition 96.
                """
                for mi in range(2):
                    sl = slice(mi * 512, (mi + 1) * 512)
                    ps = ffps.tile([P, 512], f32, tag="ffps")
                    for k2 in range(KD // 2):
                        ksl = slice(2 * k2, 2 * k2 + 2)
                        nc.tensor.matmul(
                            ps, p_pack[:, ksl, col0:col0 + P],
                            xf8[:, ksl, sl],
                            start=(k2 == 0), stop=(k2 == KD // 2 - 1),
                            skip_group_check=True, perf_mode=DR)
                        yield
                    evac_eng(dst[:, sl], ps, 1.0 / 64.0)
                    yield
                if spill is not None:
                    nc.vector.tensor_copy(out=spill,
                                          in_=dst[3 * R_ATTN:P, :])
                    yield

            def gen_qg(g):
                """qg group: G-projected queries + bq bias at ACT evac."""
                for mi in range(2):
                    sl = slice(mi * 512, (mi + 1) * 512)
                    ps = ffps.tile([P, 512], f32, tag="ffps")
                    nc.tensor.matmul(ps, gpack[:, g, :],
                                     tmpQ[:, g, sl],
                                     start=True, stop=True,
                                     skip_group_check=True)
                    yield
                    nc.vector.tensor_scalar_add(qgb[:, g, sl], ps,
                                                qbias[:, g:g + 1])
                    yield
                nc.vector.tensor_copy(out=qgb3[:, g, :],
                                      in_=qgb[3 * R_ATTN:P, g, :])
                yield

            def gen_v2nd(h):
                """V second factor for head h: vb[n, :] = tmp_v^T @ Vv."""
                g, a = divmod(h, 4)
                r0 = a * R_ATTN
                for jq in range(2):
                    ps = ffps.tile([P, 512], f32, tag="ffps")
                    for jj in range(4):
                        j = jq * 4 + jj
                        if a == 3:
                            lhsT = tmpV3[:, g, j * P:(j + 1) * P]
                            rhs = w2v[0:R_ATTN, h, :]
                        else:
                            lhsT = tmpV[r0:r0 + R_ATTN, g,
                                        j * P:(j + 1) * P]
                            rhs = w2v[r0:r0 + R_ATTN, h, :]
                        nc.tensor.matmul(
                            ps[:, jj * DH:(jj + 1) * DH], lhsT, rhs,
                            start=True, stop=True, skip_group_check=True)
                        yield
                    for jj in range(4):
                        j = jq * 4 + jj
                        nc.vector.tensor_copy(
                            out=vb[:, h, j, 0:DH],
                            in_=ps[:, jj * DH:(jj + 1) * DH])
                    yield

            # ---- prefix: K g0, Q g0, qg g0 emitted densely ----
            for gen in (gen_ff(tmpK[:, 0, :], KCOL,
                                nc.vector.tensor_scalar_mul,
                                spill=tmpK3[:, 0, :]),
                        gen_ff(tmpQ[:, 0, :], QCOL,
                               nc.vector.tensor_scalar_mul),
                        gen_qg(0)):
                for _ in gen:
                    pass

            # ---- filler queue for the head loop ----
            fillers = deque()
            fillers.append(gen_ff(tmpV[:, 0, :], VCOL,
                                  nc.vector.tensor_scalar_mul,
                                  spill=tmpV3[:, 0, :]))
            for h in range(4):
                fillers.append(gen_v2nd(h))
            fillers.append(gen_ff(tmpK[:, 1, :], KCOL + P,
                                  nc.vector.tensor_scalar_mul,
                                  spill=tmpK3[:, 1, :]))
            fillers.append(gen_ff(tmpQ[:, 1, :], QCOL + P,
                                  nc.vector.tensor_scalar_mul))
            fillers.append(gen_qg(1))
            fillers.append(gen_ff(tmpV[:, 1, :], VCOL + P,
                                  nc.vector.tensor_scalar_mul,
                                  spill=tmpV3[:, 1, :]))
            for h in range(4, 8):
                fillers.append(gen_v2nd(h))
            fillers.append(gen_ff(tmpK[:, 2, :], KCOL + 2 * P,
                                  nc.vector.tensor_scalar_mul,
                                  spill=tmpK3[:, 2, :]))
            fillers.append(gen_ff(tmpQ[:, 2, :], QCOL + 2 * P,
                                  nc.vector.tensor_scalar_mul))
            fillers.append(gen_qg(2))
            fillers.append(gen_ff(tmpV[:, 2, :], VCOL + 2 * P,
                                  nc.vector.tensor_scalar_mul,
                                  spill=tmpV3[:, 2, :]))
            for h in range(8, 12):
                fillers.append(gen_v2nd(h))

            def pump(n):
                for _ in range(n):
                    while fillers:
                        try:
                            next(fillers[0])
                            break
                        except StopIteration:
                            fillers.popleft()
                    if not fillers:
                        return

            # ---- pipelined head loop ----
            for h in range(H):
                g, a = divmod(h, 4)
                r0 = a * R_ATTN
                lag = 13 if h == 0 else 3
                prs = {}
                at = atps.tile([P, M], f32, tag="at", name=f"at{h}")
                for s in range(NPT + lag):
                    if s < NPT:
                        j = s
                        sc = scps.tile([P, M], f32, tag="sc")
                        for mi in range(2):
                            sl = slice(mi * 512, (mi + 1) * 512)
                            if a == 3:
                                lhsT = tmpK3[:, g, j * P:(j + 1) * P]
                                rhs = qgb3[:, g, sl]
                            else:
                                lhsT = tmpK[r0:r0 + R_ATTN, g,
                                            j * P:(j + 1) * P]
                                rhs = qgb[r0:r0 + R_ATTN, g, sl]
                            nc.tensor.matmul(
                                sc[:, sl], lhsT, rhs,
                                start=True, stop=True,
                                skip_group_check=True)
                        pr = prp.tile([P, M], bf16, tag="pr")
                        nc.scalar.activation(
                            out=pr, in_=sc, func=AF.Exp,
                            bias=maskb[:, j:j + 1], scale=0.125)
                        prs[j] = pr
                    pump(3 if h < 2 else 2)
                    if lag <= s < NPT + lag:
                        j = s - lag
                        pr = prs[j]
                        for mi in range(2):
                            sl = slice(mi * 512, (mi + 1) * 512)
                            nc.tensor.matmul(
                                at[0:DH + 1, sl],
                                vb[:, h, j, :], pr[:, sl],
                                start=(j == 0), stop=(j == NPT - 1),
                                skip_group_check=True)
                # normalize head h: attn_scT = at[0:64] * bcast(1/denom).
                # at is evacuated to SBUF first (the tensor engines may read
                # at most one PSUM operand per instruction, and this frees
                # the single accumulation bank quickly)
                po = DH * (h % 2)
                at_sb = recp.tile([DH + 1, M], f32, tag="atsb")
                nc.vector.tensor_copy(out=at_sb, in_=at[0:DH + 1, :])
                rec = recp.tile([1, M], bf16, tag="rec")
                with nc.allow_low_precision(
                        reason="bf16 softmax denominators feed a bf16 "
                               "broadcast matmul; 0.4% scale error is "
                               "within tolerance"):
                    nc.vector.reciprocal(out=rec, in_=at_sb[DH:DH + 1, :])
                for mi in range(2):
                    sl = slice(mi * 512, (mi + 1) * 512)
                    bc = bcps.tile([DH, 512], f32, tag="bc")
                    nc.tensor.matmul(bc, ones_row[:, 0:DH],
                                     rec[:, sl], start=True, stop=True,
                                     skip_group_check=True)
                    nc.vector.tensor_tensor(
                        out=attn_sc[h // 2][po:po + DH, sl],
                        in0=bc, in1=at_sb[0:DH, sl], op=OP.mult)
                if h == 2:
                    # residual input, not needed until the out-projection
                    for k in range(KD):
                        dma(xb[:, k, :], xb_r[:, k, :])
                if h == 5:
                    # late loads overlapping the exp marathon
                    uo = wpool.tile([P, KD, R_WO], bf16, name="uo")
                    dma(uo, uo_d.rearrange("(k p) c -> p k c", p=P))
                    vo = wpool.tile([P, 2, D], bf16, name="vo")
                    dma(vo, vo_d.rearrange("(k p) c -> p k c", p=P))
                if h == 8:
                    u1 = wpool.tile([P, KD, R_FF], bf16, name="u1")
                    dma(u1, u1_d.rearrange("(k p) c -> p k c", p=P))
                    v1 = wpool.tile([P, 2, DFF], fp8, name="v1")
                    dma(v1, v1_d.rearrange("(k p) c -> p k c", p=P))
                    u2 = wpool.tile([P, FFT, R_FF], fp8, name="u2")
                    dma(u2, u2_d.rearrange("(k p) c -> p k c", p=P))
                    v2 = wpool.tile([P, 2, D], bf16, name="v2")
                    dma(v2, v2_d.rearrange("(k p) c -> p k c", p=P))
            pump(10 ** 6)  # drain leftover fillers

        # ================= out-projection + residual =================
        z1p = ExitStack()
        with z1p:
            z1pool = z1p.enter_context(tc.tile_pool(name="z1pool", bufs=1))
            zb1 = [z1pool.tile([P, M], bf16, name=f"zb1_{k}")
                   for k in range(KD)]
            with ExitStack() as opj:
                oppool = opj.enter_context(
                    tc.tile_pool(name="oppool", bufs=1))
                h1b = oppool.tile([P, 2, M], bf16, name="h1b")
                with tc.tile_pool(name="h1ps", bufs=2, space="PSUM") as hps, \
                     tc.tile_pool(name="z1ps", bufs=2, space="PSUM") as zps:
                    for pt in range(2):
                        for mi in range(2):
                            ps = hps.tile([P, 512], f32, tag="h1")
                            for k in range(KD):
                                nc.tensor.matmul(
                                    ps, uo[:, k, pt * P:(pt + 1) * P],
                                    attn_sc[k][:, mi * 512:(mi + 1) * 512],
                                    start=(k == 0), stop=(k == KD - 1))
                            sl = slice(mi * 512, (mi + 1) * 512)
                            if mi == 0:
                                nc.vector.tensor_copy(out=h1b[:, pt, sl],
                                                      in_=ps)
                            else:
                                nc.scalar.copy(out=h1b[:, pt, sl], in_=ps)
                    for k in range(KD):
                        ps = zps.tile([P, M], f32, tag="z1")
                        for r in range(2):
                            for mi in range(2):
                                sl = slice(mi * 512, (mi + 1) * 512)
                                nc.tensor.matmul(
                                    ps[:, sl], vo[:, r, k * P:(k + 1) * P],
                                    h1b[:, r, sl],
                                    start=(r == 0), stop=(r == 1),
                                    skip_group_check=True)
                        nc.vector.scalar_tensor_tensor(
                            out=zb1[k], in0=ps, scalar=boc[:, k:k + 1],
                            in1=xb[:, k, :], op0=OP.add, op1=OP.add)

            # ---- LN1 fused with the FFN mid projection ----
            # mid = U1^T x1 = a*(U1^T zb) + c*(-colsum(U1)): the per-column
            # LN scalars commute through the contraction, so the mid matmuls
            # run on the pre-norm zb during the stats chain, and the x1b
            # apply drops off the critical path into the dff shadow.
            with ExitStack() as ln1s:
                lnp1 = ln1s.enter_context(tc.tile_pool(name="lnp1", bufs=1))
                no_aff1 = aff.get("g1") is None and aff.get("b1") is None
                bc1sb = lnp1.tile([P, 2 * M], bf16, name="bc1sb")
                with tc.tile_pool(name="midps", bufs=4,
                                  space="PSUM") as mps:
                    ac1 = _ln_stats(nc, tc, mybir, zb1, lnp1,
                                    ones_col, ones_row, eps_t)
                    mid_ps = {}
                    if no_aff1:
                        for pt in range(2):
                            for mi in range(2):
                                ps = mps.tile([P, 512], f32, tag="mid")
                                for k in range(KD):
                                    nc.tensor.matmul(
                                        ps, u1[:, k, pt * P:(pt + 1) * P],
                                        zb1[k][:, mi * 512:(mi + 1) * 512],
                                        start=(k == 0),
                                        stop=(k == KD - 1))
                                mid_ps[(pt, mi)] = ps
                    with tc.tile_pool(name="bc1ps", bufs=1,
                                      space="PSUM") as bcp1:
                        bc1 = _ln_bcast(nc, mybir, bcp1, ones_row, ac1)
                        nc.vector.tensor_copy(out=bc1sb[:, 0:M],
                                              in_=bc1[:, 0:M])
                        nc.vector.tensor_copy(out=bc1sb[:, M:2 * M],
                                              in_=bc1[:, M:2 * M])
                        if no_aff1:
                            for pt in range(2):
                                for mi in range(2):
                                    sl = slice(mi * 512, (mi + 1) * 512)
                                    sl2 = slice(M + mi * 512,
                                                M + (mi + 1) * 512)
                                    t = lnp1.tile([P, 512], f32,
                                                  tag="midt", bufs=4)
                                    nc.vector.tensor_tensor(
                                        out=t, in0=mid_ps[(pt, mi)],
                                        in1=bc1sb[:, sl], op=OP.mult)
                                    nc.vector.scalar_tensor_tensor(
                                        out=midb[:, pt, sl],
                                        in0=bc1sb[:, sl2],
                                        scalar=u1sneg[:, pt:pt + 1],
                                        in1=t, op0=OP.mult, op1=OP.add)
                # LN1 psums are closed; the x1b (residual) apply reads the
                # SBUF copies and executes in the shadow of dff
                _ln_apply(nc, mybir, zb1, bc1sb, lambda k: x1b[k], lnp1,
                          gain=aff.get("g1"), bias=aff.get("b1"))
                if not no_aff1:
                    # generic fallback: mid from the materialized x1b
                    with tc.tile_pool(name="midps2", bufs=4,
                                      space="PSUM") as mps2:
                        for pt in range(2):
                            for mi in range(2):
                                sl = slice(mi * 512, (mi + 1) * 512)
                                ps = mps2.tile([P, 512], f32, tag="mid")
                                for k in range(KD):
                                    nc.tensor.matmul(
                                        ps, u1[:, k, pt * P:(pt + 1) * P],
                                        x1b[k][:, sl],
                                        start=(k == 0),
                                        stop=(k == KD - 1))
                                (nc.vector.tensor_copy if mi == 0 else
                                 nc.scalar.copy)(out=midb[:, pt, sl],
                                                 in_=ps)

        # ================= FFN =================
        with ExitStack() as ffn:
            fpool = ffn.enter_context(tc.tile_pool(name="fpool", bufs=1))
            zb2 = [fpool.tile([P, M], bf16, name=f"zb2_{k}")
                   for k in range(KD)]
            fws = ExitStack()
            fw = fws.enter_context(tc.tile_pool(name="ffwork", bufs=1))
            dffb = fw.tile([P, FFT, M], fp8, name="dffb")
            g2b = fw.tile([P, 2, M], bf16, name="g2b")

            if b1_zero:
                # b1 == 0: process ft pairs with one [128, 2048] GELU
                # (amortizes the ACT per-op overhead)
                with tc.tile_pool(name="dffps", bufs=2,
                                  space="PSUM") as dps:
                    for fp in range(FFT // 2):
                        ps = dps.tile([P, 2, M], f32, tag="dff")
                        for sub in range(2):
                            for mi in range(2):
                                sl = slice(mi * 512, (mi + 1) * 512)
                                ft = 2 * fp + sub
                                nc.tensor.matmul(
                                    ps[:, sub, sl],
                                    v1[:, 0:2, ft * P:(ft + 1) * P],
                                    midb[:, 0:2, sl],
                                    start=True, stop=True,
                                    skip_group_check=True, perf_mode=DR)
                        # 1/64 descale of the fp8-scaled weights rides the
                        # activation's scale input
                        nc.scalar.activation(
                            out=dffb[:, 2 * fp:2 * fp + 2, :], in_=ps,
                            func=AF.Gelu, scale=1.0 / 64.0)
            else:
                with tc.tile_pool(name="dffps", bufs=3,
                                  space="PSUM") as dps:
                    for ft in range(FFT):
                        ps = dps.tile([P, M], f32, tag="dff")
                        for mi in range(2):
                            sl = slice(mi * 512, (mi + 1) * 512)
                            nc.tensor.matmul(
                                ps[:, sl], v1[:, 0:2, ft * P:(ft + 1) * P],
                                midb[:, 0:2, sl],
                                start=True, stop=True,
                                skip_group_check=True, perf_mode=DR)
                        nc.scalar.activation(
                            out=dffb[:, ft, :], in_=ps, func=AF.Gelu,
                            bias=b1c[:, ft:ft + 1], scale=1.0 / 64.0)

            with tc.tile_pool(name="g2ps", bufs=4, space="PSUM") as gps, \
                 tc.tile_pool(name="yps", bufs=2, space="PSUM") as yps:
                for pt in range(2):
                    pss = [gps.tile([P, 512], f32, tag="g2",
                                    name=f"g2_{pt}_{i}") for i in range(2)]
                    for ft2 in range(FFT // 2):
                        fsl = slice(2 * ft2, 2 * ft2 + 2)
                        for mi in range(2):
                            nc.tensor.matmul(
                                pss[mi], u2[:, fsl, pt * P:(pt + 1) * P],
                                dffb[:, fsl, mi * 512:(mi + 1) * 512],
                                start=(ft2 == 0),
                                stop=(ft2 == FFT // 2 - 1),
                                perf_mode=DR)
                    for mi in range(2):
                        sl = slice(mi * 512, (mi + 1) * 512)
                        nc.vector.tensor_scalar_mul(
                            g2b[:, pt, sl], pss[mi], 1.0 / 64.0)
                for k in range(KD):
                    ps = yps.tile([P, M], f32, tag="y")
                    for r in range(2):
                        for mi in range(2):
                            sl = slice(mi * 512, (mi + 1) * 512)
                            nc.tensor.matmul(
                                ps[:, sl], v2[:, r, k * P:(k + 1) * P],
                                g2b[:, r, sl],
                                start=(r == 0), stop=(r == 1),
                                skip_group_check=True)
                    nc.vector.scalar_tensor_tensor(
                        out=zb2[k], in0=ps, scalar=b2c[:, k:k + 1],
                        in1=x1b[k], op0=OP.add, op1=OP.add)

            fws.close()

            # ---- LN2 + store (bf16 output; host casts to f32) ----
            with tc.tile_pool(name="outp", bufs=6) as out_pool, \
                 ExitStack() as ln2s:
                lnp2 = ln2s.enter_context(tc.tile_pool(name="lnp2", bufs=1))
                bc2sb = lnp2.tile([P, 2 * M], bf16, name="bc2sb")
                ac2 = _ln_stats(nc, tc, mybir, zb2, lnp2,
                                ones_col, ones_row, eps_t)
                with tc.tile_pool(name="bc2ps", bufs=1,
                                  space="PSUM") as bcp2:
                    bc2 = _ln_bcast(nc, mybir, bcp2, ones_row, ac2)
                    nc.vector.tensor_copy(out=bc2sb[:, 0:M],
                                          in_=bc2[:, 0:M])
                    nc.vector.tensor_copy(out=bc2sb[:, M:2 * M],
                                          in_=bc2[:, M:2 * M])
                out_tiles = {}

                def ln2_out(k):
                    t = out_pool.tile([P, M], bf16, tag="out",
                                      name=f"out_{k}")
                    out_tiles[k] = t
                    return t

                def _out_half(k, mi, dst):
                    if mi == 1:
                        dma(out_d[k * P:(k + 1) * P, :], dst)

                _ln_apply(nc, mybir, zb2, bc2sb, ln2_out, lnp2,
                          gain=aff.get("g2"), bias=aff.get("b2"),
                          on_half=_out_half)


def _ln_stats(nc, tc, mybir, zb, lnp, ones_col, ones_row, eps_t):
    """Shared LN stats on prebuilt bf16 zb tiles: PE column-sum stats,
    a=rsqrt(var+eps), c=mu*a.  Returns ac [1, 2M] bf16 = [a | c].  The
    s1/s2 stat psums live in an inner scope so their banks free before the
    caller allocates the broadcast psum."""
    from contextlib import ExitStack
    OP = mybir.AluOpType
    AF = mybir.ActivationFunctionType
    f32 = mybir.dt.float32
    bf16 = mybir.dt.bfloat16

    mu = lnp.tile([1, M], f32, tag="mu")
    musq = lnp.tile([1, M], f32, tag="musq")
    var = lnp.tile([1, M], f32, tag="var")
    sd = lnp.tile([1, M], f32, tag="sd")
    a32 = lnp.tile([1, M], f32, tag="a32")
    cc = lnp.tile([1, M], f32, tag="cc")
    ac = lnp.tile([1, 2 * M], bf16, tag="ac")

    with tc.tile_pool(name="lnsps", bufs=1, space="PSUM") as lnps:
        s1 = lnps.tile([1, M], f32, tag="s1")
        s2 = lnps.tile([1, M], f32, tag="s2")
        for k in range(KD):
            zqk = lnp.tile([P, M], bf16, tag="zq", bufs=3)
            nc.vector.tensor_tensor(out=zqk, in0=zb[k], in1=zb[k],
                                    op=OP.mult)
            for mi in range(2):
                sl = slice(mi * 512, (mi + 1) * 512)
                nc.tensor.matmul(s1[:, sl], ones_col, zb[k][:, sl],
                                 start=(k == 0), stop=(k == KD - 1),
                                 skip_group_check=True)
                nc.tensor.matmul(s2[:, sl], ones_col, zqk[:, sl],
                                 start=(k == 0), stop=(k == KD - 1),
                                 skip_group_check=True)
        for mi in range(2):
            sl = slice(mi * 512, (mi + 1) * 512)
            sl2 = slice(M + mi * 512, M + (mi + 1) * 512)
            nc.vector.tensor_scalar_mul(mu[:, sl], s1[:, sl], 1.0 / D)
            nc.vector.tensor_tensor(out=musq[:, sl], in0=mu[:, sl],
                                    in1=mu[:, sl], op=OP.mult)
            nc.vector.scalar_tensor_tensor(
                out=var[:, sl], in0=s2[:, sl], scalar=1.0 / D,
                in1=musq[:, sl], op0=OP.mult, op1=OP.subtract)
            # a = 1/sqrt(var+eps): Sqrt on ACT, reciprocal on DVE
            nc.scalar.activation(out=sd[:, sl], in_=var[:, sl],
                                 func=AF.Sqrt, bias=eps_t, scale=1.0)
            nc.vector.reciprocal(out=a32[:, sl], in_=sd[:, sl])
            nc.vector.tensor_tensor(out=cc[:, sl], in0=mu[:, sl],
                                    in1=a32[:, sl], op=OP.mult)
            nc.gpsimd.tensor_copy(out=ac[:, sl], in_=a32[:, sl])
            nc.gpsimd.tensor_copy(out=ac[:, sl2], in_=cc[:, sl])
    return ac


def _ln_bcast(nc, mybir, bcpool, ones_row, ac):
    """Broadcast [a | c] to all partitions via PE ones outer-product."""
    f32 = mybir.dt.float32
    bc = bcpool.tile([P, 2 * M], f32, tag="bc")
    for i in range(4):
        sl = slice(i * 512, (i + 1) * 512)
        nc.tensor.matmul(bc[:, sl], ones_row, ac[:, sl],
                         start=True, stop=True, skip_group_check=True)
    return bc


def _ln_apply(nc, mybir, zsb, bcsb, out_tiles, lnp,
              gain=None, bias=None, on_half=None):
    """x = zb*a - c (+ affine), per column-half, on the bf16 casts with an
    SBUF bf16 broadcast: every operand is 2-byte packed SBUF, so DVE runs
    these in 4x mode."""
    OP = mybir.AluOpType
    bf16 = mybir.dt.bfloat16
    dsts = {}
    for mi in range(2):
        sl = slice(mi * 512, (mi + 1) * 512)
        sl2 = slice(M + mi * 512, M + (mi + 1) * 512)
        for k in range(KD):
            if k not in dsts:
                dsts[k] = out_tiles(k)
            dst = dsts[k]
            t1 = lnp.tile([P, 512], bf16, tag="lnt1", bufs=3)
            e1 = nc.gpsimd if k in (2, 5) else nc.vector
            e2 = nc.gpsimd if k in (1, 4) else nc.vector
            # all operands SBUF bf16 -> legal on Pool, 4x on DVE
            e1.tensor_tensor(out=t1, in0=zsb[k][:, sl], in1=bcsb[:, sl],
                             op=OP.mult)
            if gain is None and bias is None:
                e2.tensor_tensor(out=dst[:, sl], in0=t1, in1=bcsb[:, sl2],
                                 op=OP.subtract)
            else:
                e2.tensor_tensor(out=t1, in0=t1, in1=bcsb[:, sl2],
                                 op=OP.subtract)
                gk = gain[:, k:k + 1] if gain is not None else 1.0
                if bias is not None:
                    bb = bias[:, k:k + 1].to_broadcast((P, 512))
                    e2.scalar_tensor_tensor(
                        out=dst[:, sl], in0=t1, scalar=gk, in1=bb,
                        op0=OP.mult, op1=OP.add)
                else:
                    e2.tensor_scalar_mul(dst[:, sl], t1, gk)
            if on_half is not None:
                on_half(k, mi, dst)


def _prep_inputs(x, mask, Pq, Vq, bq, Pk, Vk, bk, Pv, Vv, bv,
                 Uo, Vo, bo_attn, U1, V1, b1, U2, V2, b2,
                 ln1_g, ln1_b, ln2_g, ln2_b):
    """Host-side packing: per-core in_maps for the SPMD kernel."""
    # p_pack [768, 1152]: 9 col groups of 128 (K:0-2, Q:3-5, V:6-8), each
    # 4 heads x rank-32
    p_pack = np.zeros((D, 3 * NG * P), np.float32)
    for kind, Pw in enumerate((Pk, Pq, Pv)):
        for h in range(H):
            g, a = divmod(h, 4)
            c0 = kind * NG * P + g * P + a * R_ATTN
            p_pack[:, c0:c0 + R_ATTN] = Pw[h]
    # x64 scale keeps the ~0.02-sigma weights out of the fp8e4m3 subnormal
    # range; the first-factor evacuation divides it back out
    p_pack = (p_pack * 64.0).astype(FP8)

    # gpack: block-diag Vq_h @ Vk_h^T per Q group (lhsT for qg matmul)
    gpack = np.zeros((P, NG, P), np.float32)
    qbias = np.zeros((P, NG), np.float32)
    for h in range(H):
        g, a = divmod(h, 4)
        r0 = a * R_ATTN
        gpack[r0:r0 + R_ATTN, g, r0:r0 + R_ATTN] = Vq[h] @ Vk[h].T
        qbias[r0:r0 + R_ATTN, g] = Vk[h] @ bq[0, h, 0, :]
    gpack = gpack.astype(BF16)

    # head h's Vv rows live at partitions 32*(h%4).. so the V second-factor
    # matmul's lhsT/rhs share a base partition
    w2v = np.zeros((P, H, DH), np.float32)
    for h in range(H):
        a = h % 4
        r0 = (a if a < 3 else 0) * R_ATTN
        w2v[r0:r0 + R_ATTN, h, :] = Vv[h]
    w2v = w2v.astype(BF16)

    # fold bv into bo: attn includes +bv per head; it flows through Uo@Vo
    bvec = np.asarray(bv).reshape(H * DH).astype(np.float32)
    bo_eff = (np.asarray(bo_attn, np.float32)
              + (bvec @ np.asarray(Uo, np.float32))
              @ np.asarray(Vo, np.float32))

    u1sneg = np.ascontiguousarray(
        -np.asarray(U1, np.float32).sum(axis=0).reshape(2, P).T)
    cpack = np.concatenate([
        np.asarray(b1, np.float32).reshape(FFT, P).T,
        np.asarray(bo_eff, np.float32).reshape(KD, P).T,
        np.asarray(b2, np.float32).reshape(KD, P).T,
        qbias, u1sneg], axis=1)
    awpack = np.concatenate([
        gpack.reshape(P, NG * P),
        w2v.reshape(P, H * DH)], axis=1)

    common = {
        "p_pack": p_pack, "awpack": np.ascontiguousarray(awpack),
        "cpack": np.ascontiguousarray(cpack),
        "uo": Uo.astype(BF16), "vo": Vo.astype(BF16),
        "u1": U1.astype(BF16),
        "v1": (np.asarray(V1, np.float32) * 64.0).astype(FP8),
        "u2": (np.asarray(U2, np.float32) * 64.0).astype(FP8),
        "v2": V2.astype(BF16),
    }
    has_aff1 = not (np.all(ln1_g == 1.0) and np.all(ln1_b == 0.0))
    has_aff2 = not (np.all(ln2_g == 1.0) and np.all(ln2_b == 0.0))
    b1_zero = bool(np.all(np.asarray(b1) == 0.0))
    if has_aff1:
        common["lng1"] = np.ascontiguousarray(ln1_g, np.float32)
        common["lnb1"] = np.ascontiguousarray(ln1_b, np.float32)
    if has_aff2:
        common["lng2"] = np.ascontiguousarray(ln2_g, np.float32)
        common["lnb2"] = np.ascontiguousarray(ln2_b, np.float32)

    in_maps = []
    for b in range(B):
        m = dict(common)
        xt = np.ascontiguousarray(np.asarray(x[b]).T, np.float32)
        m["xb"] = xt.astype(BF16)
        m["xf8"] = xt.astype(FP8)
        m["maskb"] = np.where(mask[b] > 0, 0.0, -1e9).astype(np.float32)
        in_maps.append(m)
    return in_maps, has_aff1, has_aff2, b1_zero


def build_program_for_inputs(n_iters: int = 1, **inputs):
    """Build (or fetch cached) program + per-core in_maps, without running."""
    inputs = {k: np.asarray(v) for k, v in inputs.items()}
    in_maps, has_aff1, has_aff2, b1_zero = _prep_inputs(**inputs)
    key = (has_aff1, has_aff2, b1_zero, n_iters)
    if key not in _prog_cache:
        _prog_cache[key] = _build_program(has_aff1, has_aff2, b1_zero,
                                          n_iters)
    return _prog_cache[key], in_maps


def kernel(**inputs):
    global last_results
    nc, in_maps = build_program_for_inputs(**inputs)
    from concourse.bass_utils import run_bass_kernel_spmd
    res = run_bass_kernel_spmd(nc, in_maps, list(range(N_CORES)))
    last_results = res
    out = np.stack([np.asarray(res.results[b]["outT"]).astype(np.float32).T
                    for b in range(B)])
    return np.ascontiguousarray(out, np.float32)
